# revision 7
# baseline (speedup 1.0000x reference)
"""DilateAttention Trainium2 kernel (nn_DilateAttention).

Full inputs q,k,v: [8, 192, 56, 56] fp32. Output: [8, 56, 56, 192] fp32.
Sharded data-parallel over batch B=8 across 8 NeuronCores.

Per-core layout: channels-on-partitions, two head groups (heads 0-3 on 128
partitions, heads 4-5 on 64). Dilated-window shifts are pure free-dim AP
offsets into zero-padded k/v images. Cross-partition reductions (sum over
head_dim) and broadcasts (attn weights over head_dim) run on the PE via 0/1
selector matmuls in float32r (full-rate, ~1e-4 rounding); exp on ScalarE;
elementwise mul/add on VectorE with the reduction add-tree split across
VectorE and GpSimd.
"""

import sys

for _p in ("/opt/trn_rl_repo",):
    if _p not in sys.path:
        sys.path.insert(0, _p)

import numpy as np

B = 8
C = 192
H = W = 56
HD = 32
NH = 6  # heads
KK = 9  # kernel*kernel
SCALE = HD ** -0.5
HWPIX = H * W  # 3136
PADH, PADW = 60, 64  # padded image: rows y in [-2,58), cols x in [-4,60)
ROW0, COL0 = 2, 4  # offsets of y=0, x=0 inside padded image
SHIFTS = [(di, dj) for di in (-2, 0, 2) for dj in (-2, 0, 2)]
NROWS = KK * NH  # 54 score rows, row m = j*NH + h

_GROUPS = [(0, 128), (1, 64)]  # (group id, partitions); group g covers heads 4g..

# PE matmul dtype for the selector reduce/broadcast passes. float32r runs the
# PE at full rate (vs 1/4 for float32) at ~1.2e-4 relative rounding.
USE_FP32R = True


def _build_consts():
    """Selector constants, arranged partition-major as 2D [P, ...] arrays."""
    consts = {}
    for g, P in _GROUPS:
        # selA[g]: lhsT for score reduce: [P, 9, 54], 1 at [p, j, j*6 + g*4 + p//32]
        a = np.zeros((P, KK, NROWS), np.float32)
        for p in range(P):
            for j in range(KK):
                a[p, j, j * NH + g * 4 + p // HD] = 1.0
        consts[f"selA{g}"] = a.reshape(P, KK * NROWS)
        # selB[g]: lhsT for attn broadcast: [54, 9, P], 1 at [j*6 + g*4 + p//32, j, p]
        b = np.zeros((NROWS, KK, P), np.float32)
        for j in range(KK):
            for p in range(P):
                b[j * NH + g * 4 + p // HD, j, p] = 1.0
        consts[f"selB{g}"] = b.reshape(NROWS, KK * P)
    # selD: [54, 6] sum over j per head
    d = np.zeros((NROWS, NH), np.float32)
    for m in range(NROWS):
        d[m, m % NH] = 1.0
    consts["selD"] = d
    # selN: [6, 54] broadcast per-head value to all (j,h) rows
    n = np.zeros((NH, NROWS), np.float32)
    for m in range(NROWS):
        n[m % NH, m] = 1.0
    consts["selN"] = n
    return consts


def _pad_memset(nc, t, P):
    """Zero only the pad strips of a [P, PADH, PADW] tile."""
    nc.gpsimd.memset(t[:, 0:ROW0, :], 0.0)
    nc.gpsimd.memset(t[:, ROW0 + H :, :], 0.0)
    nc.gpsimd.memset(t[:, ROW0 : ROW0 + H, 0:COL0], 0.0)
    nc.gpsimd.memset(t[:, ROW0 : ROW0 + H, COL0 + W :], 0.0)


def build_module():
    import concourse.bacc as bacc
    import concourse.mybir as mybir
    import concourse.tile as tile

    fp32 = mybir.dt.float32
    mmdt = mybir.dt.float32r if USE_FP32R else fp32
    AL = mybir.AluOpType

    nc = bacc.Bacc("TRN2", target_bir_lowering=False, debug=False, num_devices=B)

    q_d = nc.dram_tensor("q", [C, HWPIX], fp32, kind="ExternalInput")
    k_d = nc.dram_tensor("k", [C, H, W], fp32, kind="ExternalInput")
    v_d = nc.dram_tensor("v", [C, H, W], fp32, kind="ExternalInput")
    o_d = nc.dram_tensor("o", [HWPIX, C], fp32, kind="ExternalOutput")
    consts = _build_consts()
    c_d = {
        name: nc.dram_tensor(name, list(arr.shape), mmdt, kind="ExternalInput")
        for name, arr in consts.items()
    }

    with tile.TileContext(nc) as tc:
        with (
            tc.tile_pool(name="io", bufs=2) as io_pool,
            tc.tile_pool(name="work", bufs=2) as work_pool,
            tc.tile_pool(name="tree", bufs=3) as tree_pool,
            tc.tile_pool(name="small", bufs=1) as small_pool,
        ):
            # ---- constants to SBUF
            sel_sb = {}
            for name, arr in consts.items():
                t = small_pool.tile(list(arr.shape), mmdt, tag=f"c_{name}")
                nc.sync.dma_start(t[:], c_d[name][:])
                sel_sb[name] = t

            # ---- load q and padded k
            q_sb, k_sb = {}, {}
            for g, P in _GROUPS:
                qt = io_pool.tile([P, H, W], fp32, tag="q")
                nc.sync.dma_start(
                    qt[:], q_d[g * 128 : g * 128 + P, :].rearrange("p (a b) -> p a b", a=H)
                )
                q_sb[g] = qt
                kt = io_pool.tile([P, PADH, PADW], fp32, tag="kv")
                _pad_memset(nc, kt, P)
                nc.sync.dma_start(
                    kt[:, ROW0 : ROW0 + H, COL0 : COL0 + W],
                    k_d[g * 128 : g * 128 + P, :, :],
                )
                k_sb[g] = kt

            E_sb = small_pool.tile([NROWS, HWPIX], mmdt, tag="E")

            # ---- stage A: scores. S_ps[m=(j*6+h), px] = sum_d q * k_shift
            with tc.tile_pool(name="psS", bufs=1, space="PSUM") as psS_pool:
                S_ps = psS_pool.tile([NROWS, HWPIX], fp32, tag="S")
                first = True
                for g, P in _GROUPS:
                    selA = sel_sb[f"selA{g}"].rearrange("p (j m) -> p j m", j=KK)
                    for j, (di, dj) in enumerate(SHIFTS):
                        prod = work_pool.tile([P, H, W], mmdt, tag="prod")
                        kv = k_sb[g][
                            :, ROW0 + di : ROW0 + di + H, COL0 + dj : COL0 + dj + W
                        ]
                        nc.vector.tensor_tensor(prod[:], q_sb[g][:], kv, AL.mult)
                        pflat = prod.rearrange("p a b -> p (a b)")
                        for n0 in range(0, HWPIX, 512):
                            n1 = min(n0 + 512, HWPIX)
                            nc.tensor.matmul(
                                S_ps[:, n0:n1],
                                selA[:, j, :],
                                pflat[:, n0:n1],
                                start=first,
                                stop=(g == 1 and j == KK - 1),
                            )
                        first = False

                # exp(scale * S), evacuating PSUM
                nc.scalar.activation(
                    E_sb[:], S_ps[:], mybir.ActivationFunctionType.Exp, scale=float(SCALE)
                )

            # ---- stage B: normalize E by sum over j (chunked through PSUM)
            CHB = 448  # 7 chunks of 448 = 3136
            with tc.tile_pool(name="psB", bufs=2, space="PSUM") as psB_pool:
                for n0 in range(0, HWPIX, CHB):
                    n1 = n0 + CHB
                    D_ps = psB_pool.tile([NH, CHB], fp32, tag="D")
                    nc.tensor.matmul(
                        D_ps[:], sel_sb["selD"][:], E_sb[:, n0:n1], start=True, stop=True
                    )
                    R_ch = small_pool.tile([NH, CHB], fp32, tag="R", bufs=2)
                    Rs_ch = small_pool.tile([NH, CHB], fp32, tag="Rs", bufs=2)
                    nc.vector.reciprocal_approx_accurate(R_ch[:], D_ps[:], Rs_ch[:])
                    Rr_ch = small_pool.tile([NH, CHB], mmdt, tag="Rr", bufs=2)
                    nc.vector.tensor_copy(Rr_ch[:], R_ch[:])
                    RB_ps = psB_pool.tile([NROWS, CHB], fp32, tag="RB")
                    nc.tensor.matmul(
                        RB_ps[:], sel_sb["selN"][:], Rr_ch[:], start=True, stop=True
                    )
                    nc.vector.tensor_tensor(E_sb[:, n0:n1], E_sb[:, n0:n1], RB_ps[:], AL.mult)

            # ---- load padded v (reuses k slots)
            v_sb = {}
            for g, P in _GROUPS:
                vt = io_pool.tile([P, PADH, PADW], fp32, tag="kv")
                _pad_memset(nc, vt, P)
                nc.sync.dma_start(
                    vt[:, ROW0 : ROW0 + H, COL0 : COL0 + W],
                    v_d[g * 128 : g * 128 + P, :, :],
                )
                v_sb[g] = vt

            # ---- stage C: out[(hl,d), px] = sum_j attn_bcast_j * v_shift_j
            # attn broadcast via PE into PSUM halves; DVE muls read PSUM directly.
            HALF = HWPIX // 2  # 1568 px = 28 rows
            HROWS = H // 2
            CHC = 392  # 4 matmul chunks per half, all >=256 for fp32r full rate
            acc = {}
            with tc.tile_pool(name="psC", bufs=2, space="PSUM") as psC_pool:
                for g, P in _GROUPS:
                    selB = sel_sb[f"selB{g}"].rearrange("m (j p) -> m j p", j=KK)

                    def make_prod(j, di, dj, g=g, P=P, selB=selB):
                        prod = tree_pool.tile([P, H, W], fp32, tag="prod")
                        for hi, h0 in enumerate((0, HALF)):
                            ab_ps = psC_pool.tile([P, HALF], fp32, tag="AB")
                            for n0 in range(0, HALF, 512):
                                n1 = min(n0 + 512, HALF)
                                nc.tensor.matmul(
                                    ab_ps[:, n0:n1],
                                    selB[:, j, :],
                                    E_sb[:, h0 + n0 : h0 + n1],
                                    start=True,
                                    stop=True,
                                )
                            vv = v_sb[g][
                                :,
                                ROW0 + di + hi * HROWS : ROW0 + di + (hi + 1) * HROWS,
                                COL0 + dj : COL0 + dj + W,
                            ]
                            nc.vector.tensor_tensor(
                                prod[:, hi * HROWS : (hi + 1) * HROWS, :],
                                ab_ps.rearrange("p (a b) -> p a b", a=HROWS),
                                vv,
                                AL.mult,
                            )
                        return prod.rearrange("p a b -> p (a b)")

                    def _add(eng, dst, x, y):
                        eng.tensor_tensor(dst, x, y, AL.add)

                    # two accumulation chains: DVE takes j=0..4, GpSimd j=5..8
                    acc_g = work_pool.tile([P, HWPIX], fp32, tag="acc")
                    c2 = work_pool.tile([P, HWPIX], fp32, tag="c2")
                    pend_d = pend_g = None
                    for j, (di, dj) in enumerate(SHIFTS):
                        pf = make_prod(j, di, dj)
                        if j == 0:
                            pend_d = pf
                        elif j == 1:
                            _add(nc.vector, acc_g[:], pend_d, pf)
                        elif j <= 4:
                            _add(nc.vector, acc_g[:], acc_g[:], pf)
                        elif j == 5:
                            pend_g = pf
                        elif j == 6:
                            _add(nc.gpsimd, c2[:], pend_g, pf)
                        else:
                            _add(nc.gpsimd, c2[:], c2[:], pf)
                    _add(nc.vector, acc_g[:], acc_g[:], c2[:])
                    acc[g] = acc_g

            # ---- output: 32x32 block transpose + strided DMA to [px, C]
            for g, P in _GROUPS:
                t_sb = tree_pool.tile([P, HWPIX], fp32, tag="prod")
                nc.vector.transpose(t_sb[:], acc[g][:])
                for bc in range(P // 32):
                    c0 = g * 128 + bc * 32
                    src = t_sb[bc * 32 : (bc + 1) * 32, :].rearrange(
                        "p (bp ci) -> p bp ci", ci=32
                    )
                    dst = o_d.ap().rearrange("(bp pi) c -> pi bp c", pi=32)[:, :, c0 : c0 + 32]
                    nc.sync.dma_start(dst, src)

    nc.compile()
    return nc, consts


_CACHE = {}


def _get_module():
    if "nc" not in _CACHE:
        _CACHE["nc"], _CACHE["consts"] = build_module()
    return _CACHE["nc"], _CACHE["consts"]


def make_in_maps(q, k, v, consts):
    in_maps = []
    for b in range(B):
        m = {
            "q": np.ascontiguousarray(q[b].reshape(C, HWPIX)),
            "k": np.ascontiguousarray(k[b].reshape(C, H, W)),
            "v": np.ascontiguousarray(v[b].reshape(C, H, W)),
        }
        m.update(consts)
        in_maps.append(m)
    return in_maps


def kernel(q: np.ndarray, k: np.ndarray, v: np.ndarray) -> np.ndarray:
    from concourse import bass_utils

    nc, consts = _get_module()
    in_maps = make_in_maps(np.asarray(q), np.asarray(k), np.asarray(v), consts)
    res = bass_utils.run_bass_kernel_spmd(nc, in_maps, core_ids=list(range(B)))
    out = np.stack([r["o"].reshape(H, W, C) for r in res.results])
    return out


# revision 13
# speedup vs baseline: 1.0305x; 1.0305x over previous
"""DilateAttention Trainium2 kernel (nn_DilateAttention).

Full inputs q,k,v: [8, 192, 56, 56] fp32. Output: [8, 56, 56, 192] fp32.
Sharded data-parallel over batch B=8 across 8 NeuronCores.

Per-core layout: channels-on-partitions, two head groups (heads 0-3 on 128
partitions, heads 4-5 on 64). Dilated-window shifts are pure free-dim AP
offsets into zero-padded k/v images. Cross-partition reductions (sum over
head_dim) and broadcasts (attn weights over head_dim) run on the PE via 0/1
selector matmuls in float32r (full-rate, ~1e-4 rounding); exp on ScalarE;
elementwise mul/add on VectorE with the reduction add-tree split across
VectorE and GpSimd.
"""

import sys

for _p in ("/opt/trn_rl_repo",):
    if _p not in sys.path:
        sys.path.insert(0, _p)

import numpy as np

B = 8
C = 192
H = W = 56
HD = 32
NH = 6  # heads
KK = 9  # kernel*kernel
SCALE = HD ** -0.5
HWPIX = H * W  # 3136
PADH, PADW = 60, 60  # padded image: rows y in [-2,58), cols x in [-2,58)
ROW0, COL0 = 2, 2  # offsets of y=0, x=0 inside padded image
SHIFTS = [(di, dj) for di in (-2, 0, 2) for dj in (-2, 0, 2)]
NROWS = KK * NH  # 54 score rows, row m = j*NH + h

_GROUPS = [(1, 64), (0, 128)]  # (group id, partitions); group g covers heads 4g..; small group first

# PE matmul dtype for the selector reduce/broadcast passes. float32r runs the
# PE at full rate (vs 1/4 for float32) at ~1.2e-4 relative rounding.
USE_FP32R = True


def _build_consts():
    """Selector constants, arranged partition-major as 2D [P, ...] arrays."""
    consts = {}
    for g, P in _GROUPS:
        # selA[g]: lhsT for score reduce: [P, 9, 54], 1 at [p, j, j*6 + g*4 + p//32]
        a = np.zeros((P, KK, NROWS), np.float32)
        for p in range(P):
            for j in range(KK):
                a[p, j, j * NH + g * 4 + p // HD] = 1.0
        consts[f"selA{g}"] = a.reshape(P, KK * NROWS)
        # selB[g]: lhsT for attn broadcast: [54, 9, P], 1 at [j*6 + g*4 + p//32, j, p]
        b = np.zeros((NROWS, KK, P), np.float32)
        for j in range(KK):
            for p in range(P):
                b[j * NH + g * 4 + p // HD, j, p] = 1.0
        consts[f"selB{g}"] = b.reshape(NROWS, KK * P)
    # selD: [54, 6] sum over j per head
    d = np.zeros((NROWS, NH), np.float32)
    for m in range(NROWS):
        d[m, m % NH] = 1.0
    consts["selD"] = d
    # selN: [6, 54] broadcast per-head value to all (j,h) rows
    n = np.zeros((NH, NROWS), np.float32)
    for m in range(NROWS):
        n[m % NH, m] = 1.0
    consts["selN"] = n
    return consts


def _pad_memset(nc, t, P):
    """Zero only the pad strips of a [P, PADH, PADW] tile."""
    nc.gpsimd.memset(t[:, 0:ROW0, :], 0.0)
    nc.gpsimd.memset(t[:, ROW0 + H :, :], 0.0)
    nc.gpsimd.memset(t[:, ROW0 : ROW0 + H, 0:COL0], 0.0)
    nc.gpsimd.memset(t[:, ROW0 : ROW0 + H, COL0 + W :], 0.0)


def build_module():
    import concourse.bacc as bacc
    import concourse.mybir as mybir
    import concourse.tile as tile

    fp32 = mybir.dt.float32
    mmdt = mybir.dt.float32r if USE_FP32R else fp32
    AL = mybir.AluOpType

    nc = bacc.Bacc("TRN2", target_bir_lowering=False, debug=False, num_devices=B)

    q_d = nc.dram_tensor("q", [C, HWPIX], fp32, kind="ExternalInput")
    k_d = nc.dram_tensor("k", [C, H, W], fp32, kind="ExternalInput")
    v_d = nc.dram_tensor("v", [C, H, W], fp32, kind="ExternalInput")
    o_d = nc.dram_tensor("o", [HWPIX, C], fp32, kind="ExternalOutput")
    consts = _build_consts()
    c_d = {
        name: nc.dram_tensor(name, list(arr.shape), mmdt, kind="ExternalInput")
        for name, arr in consts.items()
    }

    with tile.TileContext(nc) as tc:
        with (
            tc.tile_pool(name="io", bufs=2) as io_pool,
            tc.tile_pool(name="work", bufs=2) as work_pool,
            tc.tile_pool(name="tree", bufs=3) as tree_pool,
            tc.tile_pool(name="small", bufs=1) as small_pool,
        ):
            # ---- constants to SBUF
            sel_sb = {}
            for name, arr in consts.items():
                t = small_pool.tile(list(arr.shape), mmdt, tag=f"c_{name}")
                nc.scalar.dma_start(t[:], c_d[name][:])
                sel_sb[name] = t

            # ---- load q and padded k
            q_sb, k_sb = {}, {}
            for g, P in _GROUPS:
                qt = io_pool.tile([P, H, W], fp32, tag="q")
                nc.scalar.dma_start(
                    qt[:], q_d[g * 128 : g * 128 + P, :].rearrange("p (a b) -> p a b", a=H)
                )
                q_sb[g] = qt
                kt = io_pool.tile([P, PADH, PADW], fp32, tag="kv")
                _pad_memset(nc, kt, P)
                nc.sync.dma_start(
                    kt[:, ROW0 : ROW0 + H, COL0 : COL0 + W],
                    k_d[g * 128 : g * 128 + P, :, :],
                )
                k_sb[g] = kt

            E_sb = small_pool.tile([NROWS, HWPIX], mmdt, tag="E")

            # ---- stage A: scores. S_ps[m=(j*6+h), px] = sum_d q * k_shift
            with tc.tile_pool(name="psS", bufs=1, space="PSUM") as psS_pool:
                S_ps = psS_pool.tile([NROWS, HWPIX], fp32, tag="S")
                first = True
                for gi, (g, P) in enumerate(_GROUPS):
                    selA = sel_sb[f"selA{g}"].rearrange("p (j m) -> p j m", j=KK)
                    for j, (di, dj) in enumerate(SHIFTS):
                        prod = work_pool.tile([P, H, W], mmdt, tag="prod", bufs=3)
                        kv = k_sb[g][
                            :, ROW0 + di : ROW0 + di + H, COL0 + dj : COL0 + dj + W
                        ]
                        a_eng = nc.gpsimd if (g, j) in ((1, 3), (1, 6), (0, 2), (0, 5)) else nc.vector
                        a_eng.tensor_tensor(prod[:], q_sb[g][:], kv, AL.mult)
                        pflat = prod.rearrange("p a b -> p (a b)")
                        for n0 in range(0, HWPIX, 512):
                            n1 = min(n0 + 512, HWPIX)
                            nc.tensor.matmul(
                                S_ps[:, n0:n1],
                                selA[:, j, :],
                                pflat[:, n0:n1],
                                start=first,
                                stop=(gi == len(_GROUPS) - 1 and j == KK - 1),
                            )
                        first = False

                # exp(scale * S), evacuating PSUM
                nc.scalar.activation(
                    E_sb[:], S_ps[:], mybir.ActivationFunctionType.Exp, scale=float(SCALE)
                )

            # ---- stage B: normalize E by sum over j (chunked through PSUM)
            CHB = 448  # 7 chunks of 448 = 3136
            with tc.tile_pool(name="psB", bufs=2, space="PSUM") as psB_pool:
                for n0 in range(0, HWPIX, CHB):
                    n1 = n0 + CHB
                    D_ps = psB_pool.tile([NH, CHB], fp32, tag="D")
                    nc.tensor.matmul(
                        D_ps[:], sel_sb["selD"][:], E_sb[:, n0:n1], start=True, stop=True
                    )
                    R_ch = small_pool.tile([NH, CHB], fp32, tag="R", bufs=2)
                    Rs_ch = small_pool.tile([NH, CHB], fp32, tag="Rs", bufs=2)
                    nc.vector.reciprocal_approx_accurate(R_ch[:], D_ps[:], Rs_ch[:])
                    Rr_ch = small_pool.tile([NH, CHB], mmdt, tag="Rr", bufs=2)
                    nc.vector.tensor_copy(Rr_ch[:], R_ch[:])
                    RB_ps = psB_pool.tile([NROWS, CHB], fp32, tag="RB")
                    nc.tensor.matmul(
                        RB_ps[:], sel_sb["selN"][:], Rr_ch[:], start=True, stop=True
                    )
                    nc.vector.tensor_tensor(E_sb[:, n0:n1], E_sb[:, n0:n1], RB_ps[:], AL.mult)

            # ---- load padded v (reuses k slots)
            v_sb = {}
            for g, P in _GROUPS:
                vt = io_pool.tile([P, PADH, PADW], fp32, tag="kv")
                _pad_memset(nc, vt, P)
                veng = nc.scalar if g == 1 else nc.sync
                veng.dma_start(
                    vt[:, ROW0 : ROW0 + H, COL0 : COL0 + W],
                    v_d[g * 128 : g * 128 + P, :, :],
                )
                v_sb[g] = vt

            # ---- stage C: out[(hl,d), px] = sum_j attn_bcast_j * v_shift_j
            # attn broadcast via PE into PSUM halves; DVE muls read PSUM directly.
            HALF = HWPIX // 2  # 1568 px = 28 rows
            HROWS = H // 2
            CHC = 392  # 4 matmul chunks per half, all >=256 for fp32r full rate
            acc = {}
            with tc.tile_pool(name="psC", bufs=2, space="PSUM") as psC_pool:
                for g, P in _GROUPS:
                    selB = sel_sb[f"selB{g}"].rearrange("m (j p) -> m j p", j=KK)

                    def make_prod(j, di, dj, g=g, P=P, selB=selB):
                        prod = tree_pool.tile([P, H, W], fp32, tag="prod")
                        for hi, h0 in enumerate((0, HALF)):
                            ab_ps = psC_pool.tile([P, HALF], fp32, tag="AB")
                            for n0 in range(0, HALF, 512):
                                n1 = min(n0 + 512, HALF)
                                nc.tensor.matmul(
                                    ab_ps[:, n0:n1],
                                    selB[:, j, :],
                                    E_sb[:, h0 + n0 : h0 + n1],
                                    start=True,
                                    stop=True,
                                )
                            vv = v_sb[g][
                                :,
                                ROW0 + di + hi * HROWS : ROW0 + di + (hi + 1) * HROWS,
                                COL0 + dj : COL0 + dj + W,
                            ]
                            nc.vector.tensor_tensor(
                                prod[:, hi * HROWS : (hi + 1) * HROWS, :],
                                ab_ps.rearrange("p (a b) -> p a b", a=HROWS),
                                vv,
                                AL.mult,
                            )
                        return prod.rearrange("p a b -> p (a b)")

                    def _add(eng, dst, x, y):
                        eng.tensor_tensor(dst, x, y, AL.add)

                    # two accumulation chains: GpSimd takes j=0..3 (finishes
                    # early, no tail), DVE takes j=4..8 + the final combine
                    acc_g = work_pool.tile([P, HWPIX], fp32, tag="acc")
                    c2 = work_pool.tile([P, HWPIX], fp32, tag="c2", bufs=1)
                    pend_d = pend_g = None
                    for j, (di, dj) in enumerate(SHIFTS):
                        pf = make_prod(j, di, dj)
                        if j == 0:
                            pend_g = pf
                        elif j == 1:
                            _add(nc.gpsimd, c2[:], pend_g, pf)
                        elif j <= 4:
                            _add(nc.gpsimd, c2[:], c2[:], pf)
                        elif j == 5:
                            pend_d = pf
                        elif j == 6:
                            _add(nc.vector, acc_g[:], pend_d, pf)
                        else:
                            _add(nc.vector, acc_g[:], acc_g[:], pf)
                    _add(nc.vector, acc_g[:], acc_g[:], c2[:])
                    # transpose + output DMA immediately per group
                    t_sb = tree_pool.tile([P, HWPIX], fp32, tag="prod")
                    nc.vector.transpose(t_sb[:], acc_g[:])
                    for bc in range(P // 32):
                        c0 = g * 128 + bc * 32
                        src_ap = t_sb[bc * 32 : (bc + 1) * 32, :].rearrange(
                            "p (bp ci) -> p bp ci", ci=32
                        )
                        dst = o_d.ap().rearrange("(bp pi) c -> pi bp c", pi=32)[
                            :, :, c0 : c0 + 32
                        ]
                        (nc.sync if bc % 2 == 0 else nc.scalar).dma_start(dst, src_ap)
                    acc[g] = acc_g


    nc.compile()
    return nc, consts


_CACHE = {}


def _get_module():
    if "nc" not in _CACHE:
        _CACHE["nc"], _CACHE["consts"] = build_module()
    return _CACHE["nc"], _CACHE["consts"]


def make_in_maps(q, k, v, consts):
    in_maps = []
    for b in range(B):
        m = {
            "q": np.ascontiguousarray(q[b].reshape(C, HWPIX)),
            "k": np.ascontiguousarray(k[b].reshape(C, H, W)),
            "v": np.ascontiguousarray(v[b].reshape(C, H, W)),
        }
        m.update(consts)
        in_maps.append(m)
    return in_maps


def kernel(q: np.ndarray, k: np.ndarray, v: np.ndarray) -> np.ndarray:
    from concourse import bass_utils

    nc, consts = _get_module()
    in_maps = make_in_maps(np.asarray(q), np.asarray(k), np.asarray(v), consts)
    res = bass_utils.run_bass_kernel_spmd(nc, in_maps, core_ids=list(range(B)))
    out = np.stack([r["o"].reshape(H, W, C) for r in res.results])
    return out


# revision 20
# speedup vs baseline: 1.1819x; 1.1469x over previous
"""DilateAttention Trainium2 kernel (nn_DilateAttention).

Full inputs q,k,v: [8, 192, 56, 56] fp32. Output: [8, 56, 56, 192] fp32.
Sharded data-parallel over batch B=8 across 8 NeuronCores.

Per-core layout: channels-on-partitions. Head group G0 (heads 0-3) fills 128
partitions directly. Group G1 (heads 4-5, 64 channels) is PIXEL-SPLIT: the
image's two halves (28 rows each, with halo) are stacked on partitions
0-63 / 64-127, so every vector op runs at full 128-lane width.

Dilated-window shifts are pure free-dim AP offsets into zero-padded k/v
images. Cross-partition reductions (sum over head_dim) and broadcasts (attn
weights over head_dim) run on the PE via 0/1 selector matmuls in float32r
(full rate, ~1.2e-4 rounding); exp on ScalarE; elementwise mul/add split
across VectorE and GpSimd.
"""

import sys

for _p in ("/opt/trn_rl_repo",):
    if _p not in sys.path:
        sys.path.insert(0, _p)

import numpy as np

B = 8
C = 192
H = W = 56
HD = 32
NH = 6  # heads
KK = 9  # kernel*kernel
SCALE = HD ** -0.5
HWPIX = H * W  # 3136
HALF = HWPIX // 2  # 1568
HROWS = H // 2  # 28
SHIFTS = [(di, dj) for di in (-2, 0, 2) for dj in (-2, 0, 2)]
NROWS = KK * 12  # 108 score rows, row m = j*12 + h*2 + half

# G0 padded image geometry: rows y in [-2,58), cols x in [-2,58)
PADH = PADW = 60
ROW0 = COL0 = 2
# G1 dup geometry: [128, 32, 60]; lower p<64: y in [-2,30); upper: y in [26,58)
PADH1 = 32

USE_FP32R = True


def _build_consts():
    """Selector constants for the [108, 1568] score layout.

    Score row m = j*12 + h*2 + half  (j in [0,9), h in [0,6), half in {0,1}).
    """
    consts = {}
    NR = 12 * KK  # 108
    # selA0lo/hi: [128, 9, 108] lhsT for G0 score reduce (per pixel-half)
    for half in (0, 1):
        a = np.zeros((128, KK, NR), np.float32)
        for p in range(128):
            for j in range(KK):
                a[p, j, j * 12 + (p // HD) * 2 + half] = 1.0
        consts[f"selA0h{half}"] = a.reshape(128, KK * NR)
    # selA1: [128, 9, 108] for the G1 dup prod (half encoded in partition)
    a = np.zeros((128, KK, NR), np.float32)
    for p in range(128):
        hh = (4 + (p % 64) // HD) * 2 + p // 64
        for j in range(KK):
            a[p, j, j * 12 + hh] = 1.0
    consts["selA1"] = a.reshape(128, KK * NR)
    # selB0lo/hi: [108, 9, 128] lhsT for G0 attn broadcast
    for half in (0, 1):
        b = np.zeros((NR, KK, 128), np.float32)
        for j in range(KK):
            for p in range(128):
                b[j * 12 + (p // HD) * 2 + half, j, p] = 1.0
        consts[f"selB0h{half}"] = b.reshape(NR, KK * 128)
    # selB1: [108, 9, 128] attn broadcast for G1 dup (half from partition)
    b = np.zeros((NR, KK, 128), np.float32)
    for j in range(KK):
        for p in range(128):
            b[j * 12 + (4 + (p % 64) // HD) * 2 + p // 64, j, p] = 1.0
    consts["selB1"] = b.reshape(NR, KK * 128)
    # selD: [108, 12] sum over j per (head, half)
    d = np.zeros((NR, 12), np.float32)
    for m in range(NR):
        d[m, m % 12] = 1.0
    consts["selD"] = d
    # selN: [12, 108] broadcast per-(head,half) value to all j rows
    n = np.zeros((12, NR), np.float32)
    for m in range(NR):
        n[m % 12, m] = 1.0
    consts["selN"] = n
    return consts


def _bank_chunks(c0, c1):
    """Split [c0,c1) at 512-element PSUM bank boundaries."""
    out = []
    while c0 < c1:
        nxt = min((c0 // 512 + 1) * 512, c1)
        out.append((c0, nxt))
        c0 = nxt
    return out


def build_module():
    import concourse.bacc as bacc
    import concourse.mybir as mybir
    import concourse.tile as tile

    fp32 = mybir.dt.float32
    mmdt = mybir.dt.float32r if USE_FP32R else fp32
    AL = mybir.AluOpType

    nc = bacc.Bacc("TRN2", target_bir_lowering=False, debug=False, num_devices=B)

    q_d = nc.dram_tensor("q", [C, H, W], fp32, kind="ExternalInput")
    k_d = nc.dram_tensor("k", [C, H, W], fp32, kind="ExternalInput")
    v_d = nc.dram_tensor("v", [C, H, W], fp32, kind="ExternalInput")
    o_d = nc.dram_tensor("o", [HWPIX, C], fp32, kind="ExternalOutput")
    consts = _build_consts()
    c_d = {
        name: nc.dram_tensor(name, list(arr.shape), mmdt, kind="ExternalInput")
        for name, arr in consts.items()
    }

    with tile.TileContext(nc) as tc:
        with (
            tc.tile_pool(name="io", bufs=2) as io_pool,
            tc.tile_pool(name="work", bufs=2) as work_pool,
            tc.tile_pool(name="tree", bufs=3) as tree_pool,
            tc.tile_pool(name="small", bufs=1) as small_pool,
        ):
            # ---- constants to SBUF (ACT hwdge queue)
            sel_sb = {}
            for name, arr in consts.items():
                t = small_pool.tile(list(arr.shape), mmdt, tag=f"c_{name}", name=f"c_{name}")
                nc.scalar.dma_start(t[:], c_d[name][:])
                sel_sb[name] = t

            def load_g1_dup(dst_name, src_d, eng):
                """[128, 32, 60] dup tile: lower y in [-2,30), upper y in [26,58)."""
                t = io_pool.tile([128, PADH1, PADW], fp32, tag="kv", name=dst_name)
                nc.gpsimd.memset(t[0:64, 0:ROW0, :], 0.0)
                nc.gpsimd.memset(t[64:128, 30:32, :], 0.0)
                nc.gpsimd.memset(t[:, :, 0:COL0], 0.0)
                nc.gpsimd.memset(t[:, :, COL0 + W :], 0.0)
                eng.dma_start(t[0:64, ROW0 : ROW0 + 30, COL0 : COL0 + W], src_d[128:192, 0:30, :])
                eng.dma_start(t[64:128, 0:30, COL0 : COL0 + W], src_d[128:192, 26:56, :])
                return t

            def load_g1_q(eng):
                t = io_pool.tile([128, HROWS, W], fp32, tag="q", name="q1")
                eng.dma_start(t[0:64, :, :], q_d[128:192, 0:HROWS, :])
                eng.dma_start(t[64:128, :, :], q_d[128:192, HROWS:H, :])
                return t

            def load_g0_pad(dst_name, src_d, eng):
                t = io_pool.tile([128, PADH, PADW], fp32, tag="kv", name=dst_name)
                nc.gpsimd.memset(t[:, 0:ROW0, :], 0.0)
                nc.gpsimd.memset(t[:, ROW0 + H :, :], 0.0)
                nc.gpsimd.memset(t[:, ROW0 : ROW0 + H, 0:COL0], 0.0)
                nc.gpsimd.memset(t[:, ROW0 : ROW0 + H, COL0 + W :], 0.0)
                eng.dma_start(t[:, ROW0 : ROW0 + H, COL0 : COL0 + W], src_d[0:128, :, :])
                return t

            # G1 first (smaller: compute starts sooner)
            k1 = load_g1_dup("k1", k_d, nc.sync)
            q1 = load_g1_q(nc.scalar)
            k0 = load_g0_pad("k0", k_d, nc.sync)
            q0 = io_pool.tile([128, H, W], fp32, tag="q", name="q0")
            nc.scalar.dma_start(q0[:], q_d[0:128, :, :])

            E_sb = small_pool.tile([NROWS, HALF], mmdt, tag="E")

            # ---- stage A: scores. S_ps[m=(j*12+h*2+half), px] = sum_d q*k_shift
            with tc.tile_pool(name="psS", bufs=1, space="PSUM") as psS_pool:
                S_ps = psS_pool.tile([NROWS, HALF], fp32, tag="S")
                selA0h = [
                    sel_sb["selA0h0"].rearrange("p (j m) -> p j m", j=KK),
                    sel_sb["selA0h1"].rearrange("p (j m) -> p j m", j=KK),
                ]
                selA1 = sel_sb["selA1"].rearrange("p (j m) -> p j m", j=KK)
                # G1 (dup): 9 muls [128, 28, 56]; one MM pass (half in rows)
                for j, (di, dj) in enumerate(SHIFTS):
                    prod = work_pool.tile([128, HROWS, W], mmdt, tag="prod", bufs=3, name="prod1")
                    kv = k1[:, ROW0 + di : ROW0 + di + HROWS, COL0 + dj : COL0 + dj + W]
                    a_eng = nc.gpsimd if j in (3, 6) else nc.vector
                    a_eng.tensor_tensor(prod[:], q1[:], kv, AL.mult)
                    pflat = prod.rearrange("p a b -> p (a b)")
                    for n0, n1 in _bank_chunks(0, HALF):
                        nc.tensor.matmul(
                            S_ps[:, n0:n1],
                            selA1[:, j, :],
                            pflat[:, n0:n1],
                            start=(j == 0),
                            stop=False,
                        )
                # G0: 9 muls [128, 56, 56]; two MM passes (one per pixel half)
                for j, (di, dj) in enumerate(SHIFTS):
                    prod = work_pool.tile([128, H, W], mmdt, tag="prod", bufs=3, name="prod0")
                    kv = k0[:, ROW0 + di : ROW0 + di + H, COL0 + dj : COL0 + dj + W]
                    a_eng = nc.gpsimd if j in (2, 5) else nc.vector
                    a_eng.tensor_tensor(prod[:], q0[:], kv, AL.mult)
                    pflat = prod.rearrange("p a b -> p (a b)")
                    for half in (0, 1):
                        for n0, n1 in _bank_chunks(0, HALF):
                            nc.tensor.matmul(
                                S_ps[:, n0:n1],
                                selA0h[half][:, j, :],
                                pflat[:, half * HALF + n0 : half * HALF + n1],
                                start=False,
                                stop=(j == KK - 1 and half == 1),
                            )

                # exp(scale * S), evacuating PSUM
                nc.scalar.activation(
                    E_sb[:], S_ps[:], mybir.ActivationFunctionType.Exp, scale=float(SCALE)
                )

            # ---- stage B: normalize E by sum over j (chunked through PSUM)
            CHB = 392  # 4 chunks of 392 = 1568
            with tc.tile_pool(name="psB", bufs=2, space="PSUM") as psB_pool:
                for n0 in range(0, HALF, CHB):
                    n1 = n0 + CHB
                    D_ps = psB_pool.tile([12, CHB], fp32, tag="D")
                    nc.tensor.matmul(
                        D_ps[:], sel_sb["selD"][:], E_sb[:, n0:n1], start=True, stop=True
                    )
                    R_ch = small_pool.tile([12, CHB], fp32, tag="R", bufs=2)
                    Rs_ch = small_pool.tile([12, CHB], fp32, tag="Rs", bufs=2)
                    nc.vector.reciprocal_approx_accurate(R_ch[:], D_ps[:], Rs_ch[:])
                    Rr_ch = small_pool.tile([12, CHB], mmdt, tag="Rr", bufs=2)
                    nc.vector.tensor_copy(Rr_ch[:], R_ch[:])
                    RB_ps = psB_pool.tile([NROWS, CHB], fp32, tag="RB")
                    nc.tensor.matmul(
                        RB_ps[:], sel_sb["selN"][:], Rr_ch[:], start=True, stop=True
                    )
                    nc.vector.tensor_tensor(E_sb[:, n0:n1], E_sb[:, n0:n1], RB_ps[:], AL.mult)

            # ---- load padded v (reuses k slots)
            v1 = load_g1_dup("v1", v_d, nc.scalar)
            v0 = load_g0_pad("v0", v_d, nc.sync)

            # ---- stage C + output, per group
            selB0h = [
                sel_sb["selB0h0"].rearrange("m (j p) -> m j p", j=KK),
                sel_sb["selB0h1"].rearrange("m (j p) -> m j p", j=KK),
            ]
            selB1 = sel_sb["selB1"].rearrange("m (j p) -> m j p", j=KK)

            def do_group(g, psC_pool):
                npx = HALF if g == 1 else HWPIX
                nrow = HROWS if g == 1 else H

                def make_prod(j, di, dj):
                    prod = tree_pool.tile([128, nrow, W], fp32, tag="prod", name=f"cprod{g}")
                    if g == 1:
                        ab_ps = psC_pool.tile([128, HALF], fp32, tag="AB")
                        for n0, n1 in _bank_chunks(0, HALF):
                            nc.tensor.matmul(
                                ab_ps[:, n0:n1], selB1[:, j, :],
                                E_sb[:, n0:n1], start=True, stop=True,
                            )
                        vv = v1[:, ROW0 + di : ROW0 + di + HROWS, COL0 + dj : COL0 + dj + W]
                        nc.vector.tensor_tensor(
                            prod[:], ab_ps.rearrange("p (a b) -> p a b", a=HROWS), vv, AL.mult
                        )
                    else:
                        for hi in (0, 1):
                            ab_ps = psC_pool.tile([128, HALF], fp32, tag="AB")
                            for n0, n1 in _bank_chunks(0, HALF):
                                nc.tensor.matmul(
                                    ab_ps[:, n0:n1], selB0h[hi][:, j, :],
                                    E_sb[:, n0:n1], start=True, stop=True,
                                )
                            vv = v0[
                                :,
                                ROW0 + di + hi * HROWS : ROW0 + di + (hi + 1) * HROWS,
                                COL0 + dj : COL0 + dj + W,
                            ]
                            nc.vector.tensor_tensor(
                                prod[:, hi * HROWS : (hi + 1) * HROWS, :],
                                ab_ps.rearrange("p (a b) -> p a b", a=HROWS),
                                vv,
                                AL.mult,
                            )
                    return prod.rearrange("p a b -> p (a b)")

                def _add(eng, dst, x, y):
                    eng.tensor_tensor(dst, x, y, AL.add)

                acc_g = work_pool.tile([128, npx], fp32, tag="acc", name=f"acc{g}")
                c2 = work_pool.tile([128, npx], fp32, tag="c2", bufs=1, name=f"c2_{g}")
                pend_d = pend_g = None
                for j, (di, dj) in enumerate(SHIFTS):
                    pf = make_prod(j, di, dj)
                    if j == 0:
                        pend_g = pf
                    elif j == 1:
                        _add(nc.gpsimd, c2[:], pend_g, pf)
                    elif j <= 4:
                        _add(nc.gpsimd, c2[:], c2[:], pf)
                    elif j == 5:
                        pend_d = pf
                    elif j == 6:
                        _add(nc.vector, acc_g[:], pend_d, pf)
                    else:
                        _add(nc.vector, acc_g[:], acc_g[:], pf)
                _add(nc.vector, acc_g[:], acc_g[:], c2[:])

                # transpose + output DMA
                t_sb = tree_pool.tile([128, npx], fp32, tag="prod", name=f"t{g}")
                nc.vector.transpose(t_sb[:], acc_g[:])
                o_view = o_d.ap().rearrange("(bp pi) c -> pi bp c", pi=32)
                for bc in range(4):
                    src_ap = t_sb[bc * 32 : (bc + 1) * 32, :].rearrange(
                        "p (bp ci) -> p bp ci", ci=32
                    )
                    if g == 1:
                        c0 = 128 + (bc % 2) * 32
                        pxoff = (bc // 2) * (HALF // 32)  # in bp units
                        dst = o_view[:, pxoff : pxoff + HALF // 32, c0 : c0 + 32]
                    else:
                        c0 = bc * 32
                        dst = o_view[:, :, c0 : c0 + 32]
                    (nc.sync if bc % 2 == 0 else nc.scalar).dma_start(dst, src_ap)

            with tc.tile_pool(name="psC", bufs=2, space="PSUM") as psC_pool:
                do_group(1, psC_pool)
                do_group(0, psC_pool)

    nc.compile()
    return nc, consts


_CACHE = {}


def _get_module():
    if "nc" not in _CACHE:
        _CACHE["nc"], _CACHE["consts"] = build_module()
    return _CACHE["nc"], _CACHE["consts"]


def make_in_maps(q, k, v, consts):
    in_maps = []
    for b in range(B):
        m = {
            "q": np.ascontiguousarray(q[b].reshape(C, H, W)),
            "k": np.ascontiguousarray(k[b].reshape(C, H, W)),
            "v": np.ascontiguousarray(v[b].reshape(C, H, W)),
        }
        m.update(consts)
        in_maps.append(m)
    return in_maps


def kernel(q: np.ndarray, k: np.ndarray, v: np.ndarray) -> np.ndarray:
    from concourse import bass_utils

    nc, consts = _get_module()
    in_maps = make_in_maps(np.asarray(q), np.asarray(k), np.asarray(v), consts)
    res = bass_utils.run_bass_kernel_spmd(nc, in_maps, core_ids=list(range(B)))
    out = np.stack([r["o"].reshape(H, W, C) for r in res.results])
    return out
